# revision 16
# baseline (speedup 1.0000x reference)
"""NVFP4 quantized linear (simulated) on 8 TRN2 NeuronCores.

out = dq(quant_nvfp4(x)) @ dq(quant_nvfp4(w)).T

Sharding: weight rows (out_features N=4096) split 512/core; x replicated;
outputs concatenated on host along N.

Palette rounding (e2m1 {0,.5,1,1.5,2,3,4,6} after x*6/blockmax) is fused
into ONE custom DVE op (NVFP4_Q_ANT, 7 nodes + select shim = 8 stages):
  v = x*r6; hi = Veltkamp 2-sig-bit round (C=2^22+1);
  t = select(v^2 <= 4, v, hi)
The |v|<=2 branch still needs rounding to the 0.5-grid; that happens on
the Scalar engine: q' = fp32(t + 1.5*2^22) is a single fp32 round-to-
nearest that snaps t to the 0.5-grid (fp32 ulp at 6291456 is exactly
0.5), then a second Scalar pass peels the constant off while converting
to fp16 (exact: palette values and the hi branch are fp16-exact).
Both regimes agree at the crossover; ties are measure-zero.
"""

import dataclasses
import sys

import numpy as np

if "/opt/trn_rl_repo" not in sys.path:
    sys.path.insert(0, "/opt/trn_rl_repo")

from concourse import bacc, mybir
from concourse import dve_ops as _dve_ops
import concourse.bass as bass  # noqa: F401
import concourse.tile as tile
import concourse.bass_utils as bass_utils
from concourse.dve_spec import Spec, Src0, Src1, C0, C1, One, select, sq, lower
from concourse.dve_uop import DveOpSpec

M, K, N = 1024, 4096, 4096
NCORES = 8
NLOC = N // NCORES  # 512
BS = 32
CHUNK = 2048  # K processed in 2 chunks per 128-row tile (SBUF pressure)
NBC = CHUNK // BS  # 128 blocks per chunk
SPC = CHUNK // 128  # 32 transposed k-slices per chunk
KT = K // 128  # 32 k-slices total

FP32 = mybir.dt.float32
FP16 = mybir.dt.float16
Alu = mybir.AluOpType
AX = mybir.AxisListType

C_FIX = 6291456.0  # 1.5 * 2^22: magic add rounds fp32 to 0.5-grid
C_VELT = 4194305.0  # 2^22 + 1: Veltkamp split -> 2 significant bits

_NC_CACHE = {}


def _nvfp4_ref(in0, in1, c0, c1, c2):
    f32 = np.float32
    x = np.asarray(in0, np.float32)
    r6 = np.asarray(in1, np.float32)
    if r6.shape != x.shape:
        if r6.ndim == 3:
            r6 = r6[..., 0]
        bs = x.size // r6.size
        r6 = np.repeat(r6, bs, axis=-1).reshape(x.shape)
    v = (x * r6).astype(np.float32)
    c = (v * f32(c1)).astype(np.float32)
    d = (c - v).astype(np.float32)
    hi = (c - d).astype(np.float32)
    return np.where(v * v <= np.asarray(c0, np.float32), v, hi).astype(np.float32)


def _register_nvfp4_op():
    name = "NVFP4_Q_ANT"
    if name in _dve_ops._SUB_OPCODE_FOR_NAME:
        return next(o for o in _dve_ops.OPS if o.name == name)
    _v = Src0 * Src1
    _c = _v * C1
    _d = _c - _v
    _hi = _c - _d
    _m = sq(_v) <= C0
    spec = Spec(body=select(_m, _v, _hi), reference=_nvfp4_ref)
    op = _dve_ops.DveOp(name, spec, subdim=False, uops_sha={})
    _dve_ops.OPS.append(op)
    _dve_ops.CUSTOM_DVE_SPECS[name] = spec
    row = _dve_ops._CUSTOM_DVE_ROW_BASE + len(_dve_ops.OPS) - 1
    _dve_ops._SUB_OPCODE_FOR_NAME[name] = row
    shas = {}
    for ver in ("v3",):
        s = DveOpSpec(name=name, opcode=row, uops=lower(spec, ver=ver), rd1_en=True)
        shas[ver] = s.sha(ver)
    op = dataclasses.replace(op, uops_sha=shas)
    _dve_ops.OPS[-1] = op
    _dve_ops.CUSTOM_DVE_SPECS[name] = op.spec
    return op


NVFP4_Q = _register_nvfp4_op()


def _quant_tile(nc, pools, src_rows, dqT, col0):
    """Quantize+dequantize 128 rows ([128, K] fp32 from DRAM) into the
    transposed fp16 slab dqT[:, :, col0:col0+128]."""
    io, work, small = pools
    xt = io.tile([128, K], FP32, name="xt", tag="xt")
    nc.sync.dma_start(xt, src_rows)
    for ch in range(K // CHUNK):
        xc = xt[:, ch * CHUNK : (ch + 1) * CHUNK]
        x3 = xc.rearrange("p (nb b) -> p nb b", b=BS)

        bmax = small.tile([128, NBC], FP32, name="bmax", tag="bmax")
        nc.vector.tensor_reduce(
            bmax, x3, axis=AX.X, op=Alu.max, apply_absolute_value=True
        )
        scl = small.tile([128, NBC], FP32, name="scl", tag="scl")
        nc.vector.tensor_scalar(scl, bmax, 1e-12, 1.0 / 6.0, Alu.max, Alu.mult)
        r6 = small.tile([128, NBC], FP32, name="r6", tag="r6")
        nc.vector.reciprocal_approx_fast(r6, scl)

        t = work.tile([128, CHUNK], FP32, name="t", tag="t")
        t3 = t.rearrange("p (nb b) -> p nb b", b=BS)
        r6_b = r6.unsqueeze(2).broadcast_to((128, NBC, BS))
        nc.vector._custom_dve(
            NVFP4_Q, out=t3, in0=x3, in1=r6_b, s0=4.0, s1=C_VELT
        )

        # single fp32 RN of t + 1.5*2^22 snaps t to the 0.5-grid
        qi = work.tile([128, CHUNK], FP32, name="qi", tag="qi")
        nc.scalar.activation(
            qi, t, mybir.ActivationFunctionType.Copy, bias=C_FIX, scale=1.0
        )
        q = work.tile([128, CHUNK], FP16, name="q", tag="q")
        nc.scalar.activation(
            q, qi, mybir.ActivationFunctionType.Copy, bias=-C_FIX, scale=1.0
        )
        q3 = q.rearrange("p (nb b) -> p nb b", b=BS)

        dq = work.tile([128, CHUNK], FP16, name="dq", tag="dq")
        dq3 = dq.rearrange("p (nb b) -> p nb b", b=BS)
        scl_b = scl.unsqueeze(2).broadcast_to((128, NBC, BS))
        nc.gpsimd.tensor_tensor(dq3, q3, scl_b, Alu.mult)

        nc.sync.dma_start_transpose(
            dqT[:, ch * SPC : (ch + 1) * SPC, col0 : col0 + 128], dq
        )


def _body(nc, tc, x_d, w_d, o_d):
    with (
        tc.tile_pool(name="persist", bufs=1) as persist,
        tc.tile_pool(name="io", bufs=2) as io,
        tc.tile_pool(name="work", bufs=2) as work,
        tc.tile_pool(name="small", bufs=2) as small,
        tc.tile_pool(name="psum", bufs=2, space="PSUM") as psum_pool,
    ):
        xdqT = persist.tile([128, KT, M], FP16)
        wdqT = persist.tile([128, KT, NLOC], FP16)
        pools = (io, work, small)

        for rt in range(NLOC // 128):  # 4 weight row-tiles
            _quant_tile(nc, pools, w_d[rt * 128 : (rt + 1) * 128, :], wdqT, rt * 128)
        for mt in range(M // 128):  # 8 x row-tiles, each feeds one matmul band
            _quant_tile(nc, pools, x_d[mt * 128 : (mt + 1) * 128, :], xdqT, mt * 128)
            ps = psum_pool.tile([128, NLOC], FP32, name="ps", tag="ps")
            for s in range(KT):
                nc.tensor.matmul(
                    ps,
                    xdqT[:, s, mt * 128 : (mt + 1) * 128],
                    wdqT[:, s, :],
                    start=(s == 0),
                    stop=(s == KT - 1),
                )
            ot = io.tile([128, NLOC], FP32, name="ot", tag="ot")
            nc.scalar.copy(ot, ps)
            nc.sync.dma_start(o_d[mt * 128 : (mt + 1) * 128, :], ot)


def _get_nc():
    if "nc" not in _NC_CACHE:
        nc = bacc.Bacc(
            "TRN2", target_bir_lowering=False, debug=False, num_devices=NCORES
        )
        x_d = nc.dram_tensor("x", (M, K), FP32, kind="ExternalInput").ap()
        w_d = nc.dram_tensor("w", (NLOC, K), FP32, kind="ExternalInput").ap()
        o_d = nc.dram_tensor("out", (M, NLOC), FP32, kind="ExternalOutput").ap()
        with tile.TileContext(nc) as tc:
            _body(nc, tc, x_d, w_d, o_d)
        nc.compile()
        _NC_CACHE["nc"] = nc
    return _NC_CACHE["nc"]


def kernel(x: np.ndarray, weight: np.ndarray, _trace: bool = False, **_):
    nc = _get_nc()
    x = np.ascontiguousarray(x, dtype=np.float32)
    weight = np.ascontiguousarray(weight, dtype=np.float32)
    in_maps = [
        {"x": x, "w": weight[c * NLOC : (c + 1) * NLOC]} for c in range(NCORES)
    ]
    res = bass_utils.run_bass_kernel_spmd(
        nc, in_maps, list(range(NCORES)), trace=_trace
    )
    out = np.concatenate([res.results[c]["out"] for c in range(NCORES)], axis=1)
    if _trace:
        kernel.last_result = res
    return out.astype(np.float32)
